# revision 1
# baseline (speedup 1.0000x reference)
"""Hyperbolic contrastive loss (nn_HGHypContrastiveLoss) on 8 Trainium2 NeuronCores.

Math (validated against the reference to ~1e-7 rel err):
  With L2-normalized rows f (so |f_i|^2 = 1), the Mobius-add norm collapses:
    num_sq = 2*(1-s)*den,  den = (1+c^2) - 2c*s,  s = <f_i, f_j>
    t = norm_diff = sqrt(2c*(1-s)/den),  1-t^2 = (1-c)^2/den
    logits = -dist/T = K*l,  l = ln((1-t)/(1+t)) = 2*ln(1-t) + ln(den) - 2*ln(1-c)
  dist >= 0 with equality on the diagonal, so logits_max == 0 (max row-shift is
  a no-op up to ~1e-8) and exp_logits = exp(K*l).

Sharding: rows (anchors) split across 8 cores, 1024 rows each. Each core gets the
full feature/one-hot matrices with columns ROTATED so its own row block sits at
columns [0, 1024) -- this makes the diagonal-tile position a compile-time
constant, keeping the program identical across cores (SPMD).

Device per (row-subchunk rc, col-chunk cc) tile [128 x 512]:
  PE : s = fT_rows^T @ fT_cols      (K=128 contraction)
       msum = ohT_rows^T @ ohT_cols (K=48; = pmask + smask, in {0,1,2})
  DVE: den, rden=1/den, sm=-2c*min(s,1), w=(sm+2c)*rden (>=0 exactly),
       l = 2*ln(1-t) + ln(den), lc = min(msum,1)*l (+row-sum accum)
  ACT: t=sqrt(w), ln(1-t), ln(den), e=exp(K*l - K*C2) (+row-sum accum)
  Diagonal tiles additionally extract e_ii, l_ii via identity-mask + accum.
Host: npos from label bincounts (exact), denominator = rowsum(e) - e_ii + 1e-8,
      log-prob row sums, mean over valid rows.
"""

import numpy as np

import concourse.bass as bass
import concourse.tile as tile
import concourse.mybir as mybir
from concourse.bass_utils import run_bass_kernel_spmd

F32 = mybir.dt.float32
AX = mybir.AxisListType
OP = mybir.AluOpType
AF = mybir.ActivationFunctionType

N = 8192
D = 128
NCORES = 8
RPC = N // NCORES        # 1024 rows per core
NRC = RPC // 128         # 8 row sub-chunks of 128
CCW = 512                # col chunk width
NCC = N // CCW           # 16 col chunks
NOH = 48                 # one-hot rows (32 primary + 16 secondary)

C = 0.05
SQRT_C = float(np.sqrt(C))
TEMP = 0.5
K = 1.0 / (SQRT_C * TEMP)
C2 = float(2.0 * np.log1p(-C))        # 2*ln(1-c)
DEN_B = 1.0 + C * C

_CACHE: dict = {}


class _SplitDrainTC(tile.TileContext):
    """TileContext whose kernel-tail drain is split into a chain of
    single-wait drains: the walrus CTRL encoding cannot hold the 5 sync
    waits (ACT, PE, DVE, 2 DMA queues) the stock drain carries."""

    def _drain_and_barrier(self, tick_clock, wait_clock):
        from concourse.tile import ScopedClock

        d = self.nc.sync.drain()
        wait_clock.add_sem_waits(d.ins, ScopedClock({None: tick_clock.global_clock}))
        si = d.ins.sync_info
        waits = list(si.on_wait) if si is not None else []
        if len(waits) > 1:
            si.on_wait = waits[:1]
            for w in waits[1:]:
                d2 = self.nc.sync.drain()
                si2 = d2.ins.sync_info
                if si2 is None:
                    d2.ins.sync_info = mybir.SyncInfo(on_wait=[w], on_update=[])
                else:
                    si2.on_wait = [w]
        self.nc.all_engine_barrier()
        popped = self.nc._tile_sem_poison_stack.pop()
        assert popped is self._sem_poison
        self.nc.clear_and_free_semaphores(list(self.sems.allocated().values()))
        self.nc.all_engine_barrier()


def _build_nc():
    nc = bass.Bass()
    # single input DMA (fewer DMA queues -> fewer waits on the tail drain):
    # cols [0,N) = fT on 128 partitions; cols [N,2N) = ohT on partitions [0,48)
    inp = nc.dram_tensor("inp", [D, 2 * N], F32, kind="ExternalInput")
    outs = nc.dram_tensor("outs", [128, 2 * NRC], F32, kind="ExternalOutput")

    with (
        _SplitDrainTC(nc) as tc,
        tc.tile_pool(name="const", bufs=1) as cpool,
        tc.tile_pool(name="work", bufs=3) as wpool,
        tc.tile_pool(name="acc", bufs=2) as apool,
        tc.tile_pool(name="ps", bufs=3, space="PSUM") as pspool,
        tc.tile_pool(name="pm", bufs=3, space="PSUM") as pmpool,
    ):
        inps = cpool.tile([D, 2 * N], F32)
        nc.gpsimd.dma_start(inps[:], inp[:])
        fTs = inps[:, 0:N]
        ohTs = inps[0:NOH, N:2 * N]

        bias_e = cpool.tile([128, 1], F32)
        nc.vector.memset(bias_e[:], -K * C2)

        # cols [0,NRC) = rowsum(e), cols [NRC,2*NRC) = rowsum(l*combined)
        fin = cpool.tile([128, 2 * NRC], F32)

        for rc in range(NRC):
            acc_e = apool.tile([128, NCC], F32, tag="acc_e")
            acc_lc = apool.tile([128, NCC], F32, tag="acc_lc")
            lhs_f = inps[:, rc * 128:(rc + 1) * 128]
            lhs_oh = inps[0:NOH, N + rc * 128:N + (rc + 1) * 128]
            for cc in range(NCC):
                ps = pspool.tile([128, CCW], F32, tag="ps")
                nc.tensor.matmul(ps[:], lhs_f, inps[:, cc * CCW:(cc + 1) * CCW],
                                 start=True, stop=True)
                pm = pmpool.tile([128, CCW], F32, tag="pm")
                nc.tensor.matmul(pm[:], lhs_oh, inps[0:NOH, N + cc * CCW:N + (cc + 1) * CCW],
                                 start=True, stop=True)

                den = wpool.tile([128, CCW], F32, tag="den")
                nc.vector.tensor_scalar(den[:], ps[:], -2.0 * C, DEN_B, OP.mult, OP.add)
                rden = wpool.tile([128, CCW], F32, tag="rden")
                nc.vector.reciprocal(rden[:], den[:])
                sm = wpool.tile([128, CCW], F32, tag="sm")
                nc.vector.tensor_scalar(sm[:], ps[:], 1.0, -2.0 * C, OP.min, OP.mult)
                w = wpool.tile([128, CCW], F32, tag="w")
                nc.vector.scalar_tensor_tensor(w[:], sm[:], 2.0 * C, rden[:], OP.add, OP.mult)

                t = wpool.tile([128, CCW], F32, tag="t")
                nc.scalar.activation(t[:], w[:], AF.Sqrt)
                lnq = wpool.tile([128, CCW], F32, tag="lnq")
                nc.scalar.activation(lnq[:], t[:], AF.Ln, bias=1.0, scale=-1.0)
                # ln(den) = -ln(rden); reading rden (not den) keeps den DVE-local
                # so no instruction needs two cross-engine waits (walrus allows 1).
                lnrden = wpool.tile([128, CCW], F32, tag="lnrden")
                nc.scalar.activation(lnrden[:], rden[:], AF.Ln)

                l = wpool.tile([128, CCW], F32, tag="l")
                nc.vector.scalar_tensor_tensor(l[:], lnq[:], 2.0, lnrden[:], OP.mult, OP.subtract)
                e = wpool.tile([128, CCW], F32, tag="e")
                nc.scalar.activation(e[:], l[:], AF.Exp, scale=K, bias=bias_e[:],
                                     accum_out=acc_e[:, cc:cc + 1])
                tch = wpool.tile([128, 1], F32, tag="tch")
                nc.vector.tensor_copy(tch[:], pm[:, 0:1])
                cmb = wpool.tile([128, CCW], F32, tag="cmb")
                nc.vector.tensor_scalar(cmb[:], pm[:], 1.0, None, OP.min)
                lc = wpool.tile([128, CCW], F32, tag="lc")
                nc.vector.scalar_tensor_tensor(lc[:], cmb[:], 1.0, l[:], OP.mult, OP.mult,
                                               accum_out=acc_lc[:, cc:cc + 1])

            nc.vector.reduce_sum(fin[:, rc:rc + 1], acc_e[:], axis=AX.X)
            nc.vector.reduce_sum(fin[:, NRC + rc:NRC + rc + 1], acc_lc[:], axis=AX.X)

        nc.gpsimd.dma_start(outs[:], fin[:])

    return nc


def _get_nc():
    if "nc" not in _CACHE:
        _CACHE["nc"] = _build_nc()
    return _CACHE["nc"]


def kernel(features, primary_labels, secondary_labels):
    features = np.asarray(features, dtype=np.float32)
    pl = np.asarray(primary_labels).astype(np.int64)
    sl = np.asarray(secondary_labels).astype(np.int64)

    nrm = np.maximum(np.linalg.norm(features, axis=1, keepdims=True), 1e-12)
    f = (features / nrm).astype(np.float32)
    fT = np.ascontiguousarray(f.T)                      # [128, N]

    oh = np.zeros((NOH, N), dtype=np.float32)
    oh[pl, np.arange(N)] = 1.0
    oh[32 + sl, np.arange(N)] = 1.0

    in_maps = []
    for c in range(NCORES):
        shift = c * RPC
        buf = np.zeros((D, 2 * N), dtype=np.float32)
        buf[:, 0:N] = np.roll(fT, -shift, axis=1)
        buf[0:NOH, N:2 * N] = np.roll(oh, -shift, axis=1)
        in_maps.append({"inp": buf})

    nc = _get_nc()
    res = run_bass_kernel_spmd(nc, in_maps, list(range(NCORES)))
    results = res.results

    se = np.empty(N, np.float64)
    slc = np.empty(N, np.float64)
    for c in range(NCORES):
        r = results[c]
        for rc in range(NRC):
            g0 = c * RPC + rc * 128
            se[g0:g0 + 128] = r["outs"][:, rc]
            slc[g0:g0 + 128] = r["outs"][:, NRC + rc]

    # diagonal terms mirrored on host (s_ii = |f_i|^2, fp32 math like the device)
    s_ii = np.sum(f * f, axis=1, dtype=np.float32)
    den_ii = (np.float32(DEN_B) + np.float32(-2 * C) * s_ii).astype(np.float32)
    rden_ii = (np.float32(1.0) / den_ii).astype(np.float32)
    sm_ii = (np.float32(-2 * C) * np.minimum(s_ii, np.float32(1.0))).astype(np.float32)
    w_ii = ((sm_ii + np.float32(2 * C)) * rden_ii).astype(np.float32)
    t_ii = np.sqrt(w_ii).astype(np.float32)
    ld = (np.float32(2.0) * np.log(np.float32(1.0) - t_ii) - np.log(rden_ii)).astype(np.float32)
    ed = np.exp(np.float32(K) * ld + np.float32(-K * C2)).astype(np.float32)

    cnt_p = np.bincount(pl, minlength=32)
    cnt_s = np.bincount(sl, minlength=16)
    comb = pl * 16 + sl
    cnt_ps = np.bincount(comb, minlength=512)
    npos = (cnt_p[pl] + cnt_s[sl] - cnt_ps[comb] - 1).astype(np.float64)

    denominator = se - ed + 1e-8
    S2 = K * (slc - ld - C2 * npos)
    row_sum = S2 - np.log(denominator) * npos
    valid = npos > 0
    per_row = np.where(valid, row_sum / np.maximum(npos, 1.0), 0.0)
    n_valid = valid.sum()
    loss = -per_row.sum() / max(n_valid, 1) * TEMP if n_valid > 0 else 0.0
    loss = np.nan_to_num(np.float32(loss), nan=0.0, posinf=0.0, neginf=0.0)
    return np.float32(loss)



# revision 28
# speedup vs baseline: 5.1603x; 5.1603x over previous
"""Hyperbolic contrastive loss (nn_HGHypContrastiveLoss) on 8 Trainium2 NeuronCores.

Math: with L2-normalized f (|f|=1), s = <f_i,f_j>, the Mobius/artanh chain
collapses to logits_ij = -K*acosh(x),  x = C1*s + C0X  (affine).
Using d2 := e^{-acosh(x)} = x - sqrt(x^2-1):
    exp(logits) = d2^K = exp(K*ln(d2)),   logits = K*ln(d2)
x > 1 is guaranteed by a small host-chosen shrink eta on the rhs operand so
the bf16 matmul diagonal stays strictly below 1.

Symmetry sharding: the NxN matrix is symmetric, so each core computes only
5 of 8 local 1024-column blocks (cols rotated so its own row-block is block
0). Blocks 0 (diagonal) and 4 (antipodal, computed by both pair cores)
contribute via row sums only; blocks 1-3 additionally produce per-column
sums (PE ones-matmul, accumulated over each 2-row-chunk group, row 0 of the
PSUM result DMA'd out) which the host adds to the row sums of the
corresponding other band. Elementwise work drops to 5/8.

Device per core:
  PE : s/pm matmuls (bf16), ones-matmul column sums of e and lc tiles
  DVE: evac x = C1*s + C0X (3/5 of chunks), z = x*x, d2 = x - g,
       lc = min(pm,1)*l2 (+row accum)
  ACT: evac (2/5), g = Sqrt(z - 1), l2 = Ln(d2), e = Exp(K*l2) (+row accum),
       batched per 2-row-chunk group by table set (sqrt | ln+exp)
Host: npos from label bincounts (exact); diagonal e_ii/l2_ii mirrored in
numpy with identical bf16 arithmetic; column-sum reassembly; final mean.
"""

import numpy as np
import ml_dtypes

import concourse.bass as bass
import concourse.tile as tile
import concourse.mybir as mybir
from concourse.bass_utils import run_bass_kernel_spmd

F32 = mybir.dt.float32
BF16 = mybir.dt.bfloat16
AX = mybir.AxisListType
OP = mybir.AluOpType
AF = mybir.ActivationFunctionType
BF = ml_dtypes.bfloat16

N = 8192
D = 128
NCORES = 8
RPC = N // NCORES        # 1024 rows per core
NRC = RPC // 128         # 8 row sub-chunks of 128
CCW = 512
NCP = 5                  # local 1024-col chunk-pairs computed (blocks 0-4)
NLC = NCP * 1024         # 5120 local columns
NCS = 6                  # column-sum strips of 512 (blocks 1-3)
CS0 = 1024               # first column with column sums
NOH = 48
GRC = 2                  # row sub-chunks per ACT-batching group
NGRP = NRC // GRC

C = 0.05
TEMP = 0.5
K = float(1.0 / (np.sqrt(C) * TEMP))          # 8.944271909999159
SD = 0.45125
C1 = float(np.float32(-0.1 / SD))              # x = C1*s + C0X
C0X = float(np.float32(0.1 / SD + 1.0))

# input blob layout (bf16)
OF_LF = 0
OF_RF = 1024
OF_LO = OF_RF + NLC
OF_RO = OF_LO + 1024
W_IN = OF_RO + NLC

# outs: [128, 0:8) se rowsums | [8:16) slc rowsums
# outs_cs: [1, grp*2*NCS*CCW] per-group colsums (e strips then lc strips)
W_CS = NGRP * 2 * NCS * CCW
_CACHE: dict = {}


class _SplitDrainTC(tile.TileContext):
    """TileContext whose kernel-tail drain is split into a chain of
    single-wait drains (walrus CTRL holds one sync wait)."""

    def _drain_and_barrier(self, tick_clock, wait_clock):
        from concourse.tile import ScopedClock

        d = self.nc.sync.drain()
        wait_clock.add_sem_waits(d.ins, ScopedClock({None: tick_clock.global_clock}))
        si = d.ins.sync_info
        waits = list(si.on_wait) if si is not None else []
        if len(waits) > 1:
            si.on_wait = waits[:1]
            for w in waits[1:]:
                d2 = self.nc.sync.drain()
                si2 = d2.ins.sync_info
                if si2 is None:
                    d2.ins.sync_info = mybir.SyncInfo(on_wait=[w], on_update=[])
                else:
                    si2.on_wait = [w]
        self.nc.all_engine_barrier()
        popped = self.nc._tile_sem_poison_stack.pop()
        assert popped is self._sem_poison
        self.nc.clear_and_free_semaphores(list(self.sems.allocated().values()))
        self.nc.all_engine_barrier()


def _build_nc():
    nc = bass.Bass()
    inp = nc.dram_tensor("inp", [D, W_IN], BF16, kind="ExternalInput")
    outs = nc.dram_tensor("outs", [128, 2 * NRC], F32, kind="ExternalOutput")
    outs_cs = nc.dram_tensor("outs_cs", [1, W_CS], F32, kind="ExternalOutput")

    with (
        _SplitDrainTC(nc) as tc,
        tc.tile_pool(name="const", bufs=1) as cpool,
        tc.tile_pool(name="mp", bufs=2) as mpool,
        tc.tile_pool(name="zp", bufs=2) as zpool,
        tc.tile_pool(name="gp", bufs=2) as gpool,
        tc.tile_pool(name="lp", bufs=3) as lpool,
        tc.tile_pool(name="ep", bufs=2) as epool,
        tc.tile_pool(name="lq", bufs=2) as lqpool,
        tc.tile_pool(name="acc", bufs=3) as apool,
        tc.tile_pool(name="lcw", bufs=3) as lcpool,
        tc.tile_pool(name="fen", bufs=2) as fenpool,
        tc.tile_pool(name="st", bufs=2) as stpool,
        tc.tile_pool(name="ps", bufs=2, space="PSUM") as pspool,
        tc.tile_pool(name="pm", bufs=2, space="PSUM") as pmpool,
        tc.tile_pool(name="cs", bufs=2, space="PSUM") as cspool,
    ):
        inps = cpool.tile([D, W_IN], BF16)
        nc.gpsimd.dma_start(inps[:, 0:OF_LO], inp[:, 0:OF_LO])
        nc.gpsimd.dma_start(inps[:, OF_LO:W_IN], inp[:, OF_LO:W_IN])

        bias_s = cpool.tile([128, 1], F32)
        nc.vector.memset(bias_s[:], -1.0)
        ones = cpool.tile([128, 1], BF16)
        nc.vector.memset(ones[:], 1.0)

        fin = cpool.tile([128, 2 * NRC], F32)

        def lhs_f(rc):
            return inps[:, OF_LF + rc * 128:OF_LF + (rc + 1) * 128]

        def rhs_f(c0, c1):
            return inps[:, OF_RF + c0:OF_RF + c1]

        def lhs_oh(rc):
            return inps[0:NOH, OF_LO + rc * 128:OF_LO + (rc + 1) * 128]

        def rhs_oh(c0, c1):
            return inps[0:NOH, OF_RO + c0:OF_RO + c1]

        def emit_pm_lc_cs(grp, rcs, lts, ets):
            # DVE fence on the last Ln output so lc STTs carry one wait.
            fence = fenpool.tile([128, 1], BF16, tag="fence")
            nc.vector.tensor_copy(fence[:], lts[rcs[-1]][:, 0:1])
            lqs = {}
            for rc in rcs:
                lq = lqpool.tile([128, NCS * CCW], BF16, tag="lq")
                lqs[rc] = lq
                acc_lc = apool.tile([128, 2 * NCP], F32, tag="acc_lc")
                for cc in range(2 * NCP):
                    pm = pmpool.tile([128, CCW], F32, tag="pm")
                    nc.tensor.matmul(pm[:], lhs_oh(rc),
                                     rhs_oh(cc * CCW, (cc + 1) * CCW),
                                     start=True, stop=True)
                    if 2 <= cc < 8:        # cols [1024,4096): keep product
                        lco = lq[:, (cc - 2) * CCW:(cc - 1) * CCW]
                    else:
                        lcot = lcpool.tile([128, CCW], BF16, tag="lc")
                        lco = lcot[:]
                    nc.vector.scalar_tensor_tensor(
                        lco, pm[:], 1.0,
                        lts[rc][:, cc * CCW:(cc + 1) * CCW],
                        OP.min, OP.mult,
                        accum_out=acc_lc[:, cc:cc + 1])
                nc.vector.reduce_sum(fin[:, NRC + rc:NRC + rc + 1],
                                     acc_lc[:], axis=AX.X)
            # column sums for blocks 1-3, accumulated over the group's rcs
            base = grp * 2 * NCS * CCW
            stage = stpool.tile([1, 2 * NCS * CCW], F32, tag="stage")
            for k in range(NCS):
                cse = cspool.tile([1, CCW], F32, tag="cs")
                for j, rc in enumerate(rcs):
                    nc.tensor.matmul(
                        cse[:], ones[:, 0:1],
                        ets[rc][:, CS0 + k * CCW:CS0 + (k + 1) * CCW],
                        start=(j == 0), stop=(j == len(rcs) - 1))
                nc.scalar.activation(stage[:, k * CCW:(k + 1) * CCW],
                                     cse[:], AF.Copy)
            for k in range(NCS):
                csl = cspool.tile([1, CCW], F32, tag="cs")
                for j, rc in enumerate(rcs):
                    nc.tensor.matmul(
                        csl[:], ones[:, 0:1],
                        lqs[rc][:, k * CCW:(k + 1) * CCW],
                        start=(j == 0), stop=(j == len(rcs) - 1))
                nc.vector.tensor_copy(
                    stage[:, (NCS + k) * CCW:(NCS + k + 1) * CCW], csl[:])
            nc.gpsimd.dma_start(
                outs_cs[:, base:base + 2 * NCS * CCW], stage[:])

        prev = None
        for grp in range(NGRP):
            rcs = tuple(range(grp * GRC, (grp + 1) * GRC))
            mts, zts, gts, lts, ets = {}, {}, {}, {}, {}
            for rc in rcs:
                mt = mpool.tile([128, NLC], BF16, tag="m")
                mts[rc] = mt
                for cp in range(NCP):
                    ps = pspool.tile([128, 2 * CCW], F32, tag="ps")
                    nc.tensor.matmul(ps[:, 0:CCW], lhs_f(rc),
                                     rhs_f(cp * 1024, cp * 1024 + CCW),
                                     start=True, stop=True)
                    nc.tensor.matmul(ps[:, CCW:2 * CCW], lhs_f(rc),
                                     rhs_f(cp * 1024 + CCW, (cp + 1) * 1024),
                                     start=True, stop=True)
                    dst = mt[:, cp * 1024:(cp + 1) * 1024]
                    if cp >= 3:
                        nc.scalar.activation(dst, ps[:], AF.Copy,
                                             bias=C0X, scale=C1)
                    else:
                        nc.vector.tensor_scalar(dst, ps[:], C1, C0X,
                                                OP.mult, OP.add)
                zt = zpool.tile([128, NLC], BF16, tag="z")
                zts[rc] = zt
                nc.vector.tensor_tensor(zt[:], mt[:], mt[:], OP.mult)
            if prev is not None:
                emit_pm_lc_cs(*prev)
            for rc in rcs:
                gt = gpool.tile([128, NLC], BF16, tag="g")
                gts[rc] = gt
                nc.scalar.activation(gt[:], zts[rc][:], AF.Sqrt,
                                     bias=bias_s[:])          # g = sqrt(z-1)
            for rc in rcs:       # d2 = x - g (reuse z tile)
                nc.vector.tensor_tensor(zts[rc][:], mts[rc][:], gts[rc][:],
                                        OP.subtract)
            for rc in rcs:
                lt = lpool.tile([128, NLC], BF16, tag="l")
                lts[rc] = lt
                nc.scalar.activation(lt[:], zts[rc][:], AF.Ln)
            for rc in rcs:       # e = exp(K*l2), row-accumulated
                et = epool.tile([128, NLC], BF16, tag="e")
                ets[rc] = et
                acc_e = apool.tile([128, 1], F32, tag="acc_e")
                nc.scalar.activation(et[:], lts[rc][:], AF.Exp, scale=K,
                                     accum_out=acc_e[:])
                nc.vector.tensor_copy(fin[:, rc:rc + 1], acc_e[:])
            prev = (grp, rcs, lts, ets)
        emit_pm_lc_cs(*prev)

        nc.gpsimd.dma_start(outs[:], fin[:])

    _split_multi_waits(nc)
    return nc


def _split_multi_waits(nc):
    """Walrus CTRL encodings hold a single sync wait. For any instruction
    carrying more, peel the extra waits onto same-engine drain instructions
    inserted immediately before it (same queue position -> identical
    semantics)."""
    ctr = 0
    for bbh in nc.bb_map.values():
        bb = bbh.bb if hasattr(bbh, "bb") else bbh
        il = list(bb.instructions)
        out = []
        changed = False
        for ins in il:
            si = ins.sync_info
            waits = list(si.on_wait) if si is not None else []
            if len(waits) > 1:
                changed = True
                for w in waits[1:]:
                    d = mybir.InstDrain(name=f"wsplit{ctr}", ins=[], outs=[])
                    ctr += 1
                    d.engine = ins.engine
                    d.sync_info = mybir.SyncInfo(on_wait=[w], on_update=[])
                    nc.register_instruction(d, overwrite=True)
                    out.append(d)
                si.on_wait = waits[:1]
            out.append(ins)
        if changed:
            bb.instructions = out


def _get_nc():
    if "nc" not in _CACHE:
        _CACHE["nc"] = _build_nc()
    return _CACHE["nc"]


def _bf(x):
    return np.asarray(x, dtype=BF).astype(np.float32)


def kernel(features, primary_labels, secondary_labels):
    features = np.asarray(features, dtype=np.float32)
    pl = np.asarray(primary_labels).astype(np.int64)
    sl = np.asarray(secondary_labels).astype(np.int64)

    nrm = np.maximum(np.linalg.norm(features, axis=1, keepdims=True), 1e-12)
    f = (features / nrm).astype(np.float32)

    lhs_bf = _bf(f)
    eta = 0.0
    for _ in range(6):
        rhs_bf = _bf((1.0 - eta) * f)
        diag = np.einsum("nd,nd->n", lhs_bf, rhs_bf).astype(np.float32)
        if diag.max() <= 1.0 - 3e-4:
            break
        eta = eta + (float(diag.max()) - (1.0 - 4e-4))

    lhsT = np.ascontiguousarray(lhs_bf.T).astype(BF)
    rhsT = np.ascontiguousarray(rhs_bf.T).astype(BF)

    oh = np.zeros((NOH, N), dtype=BF)
    oh[pl, np.arange(N)] = 1.0
    oh[32 + sl, np.arange(N)] = 1.0

    in_maps = []
    for c in range(NCORES):
        shift = c * RPC
        lr = np.roll(lhsT, -shift, axis=1)
        rr = np.roll(rhsT, -shift, axis=1)
        orr = np.roll(oh, -shift, axis=1)
        buf = np.zeros((D, W_IN), dtype=BF)
        buf[:, OF_LF:OF_LF + 1024] = lr[:, 0:1024]
        buf[:, OF_RF:OF_RF + NLC] = rr[:, 0:NLC]
        buf[0:NOH, OF_LO:OF_LO + 1024] = orr[:, 0:1024]
        buf[0:NOH, OF_RO:OF_RO + NLC] = orr[:, 0:NLC]
        in_maps.append({"inp": buf})

    nc = _get_nc()
    res = run_bass_kernel_spmd(nc, in_maps, list(range(NCORES)))
    results = res.results

    se = np.zeros(N, np.float64)
    slc2 = np.zeros(N, np.float64)
    for c in range(NCORES):
        r = results[c]
        for rc in range(NRC):
            g0 = c * RPC + rc * 128
            se[g0:g0 + 128] += r["outs"][:, rc]
            slc2[g0:g0 + 128] += r["outs"][:, NRC + rc]
        # column sums: local cols [1024, 4096) -> global rows of bands c+1..c+3
        cs = r["outs_cs"][0].astype(np.float64)   # [NGRP * 2 * NCS * CCW]
        cs = cs.reshape(NGRP, 2, NCS * CCW)
        cse = cs[:, 0, :].sum(axis=0)             # [3072] over row groups
        csl = cs[:, 1, :].sum(axis=0)
        jg = (np.arange(CS0, CS0 + NCS * CCW) + c * RPC) % N
        se[jg] += cse
        slc2[jg] += csl

    # diagonal mirror (device bf16 arithmetic replicated on host)
    x_ii = _bf(np.float32(C1) * diag + np.float32(C0X))
    z_ii = _bf(x_ii * x_ii)
    g_ii = _bf(np.sqrt(z_ii - np.float32(1.0)))
    d2_ii = _bf(x_ii - g_ii)
    l2_ii = _bf(np.log(d2_ii)).astype(np.float64)
    e_ii = np.exp(np.float32(K) * l2_ii.astype(np.float32)).astype(np.float64)

    cnt_p = np.bincount(pl, minlength=32)
    cnt_s = np.bincount(sl, minlength=16)
    comb = pl * 16 + sl
    cnt_ps = np.bincount(comb, minlength=512)
    npos = (cnt_p[pl] + cnt_s[sl] - cnt_ps[comb] - 1).astype(np.float64)

    denom = se - e_ii + 1e-8
    row_sum = K * (slc2 - l2_ii) - np.log(denom) * npos
    valid = npos > 0
    per_row = np.where(valid, row_sum / np.maximum(npos, 1.0), 0.0)
    n_valid = valid.sum()
    loss = -per_row.sum() / max(n_valid, 1) * TEMP if n_valid > 0 else 0.0
    loss = np.nan_to_num(np.float32(loss), nan=0.0, posinf=0.0, neginf=0.0)
    return np.float32(loss)


# revision 30
# speedup vs baseline: 5.2669x; 1.0207x over previous
"""Hyperbolic contrastive loss (nn_HGHypContrastiveLoss) on 8 Trainium2 NeuronCores.

Math: with L2-normalized f (|f|=1), s = <f_i,f_j>, the Mobius/artanh chain
collapses to logits_ij = -K*acosh(x),  x = C1*s + C0X  (affine).
Using d2 := e^{-acosh(x)} = x - sqrt(x^2-1):
    exp(logits) = d2^K = exp(K*ln(d2)),   logits = K*ln(d2)
x > 1 is guaranteed by a small host-chosen shrink eta on the rhs operand so
the bf16 matmul diagonal stays strictly below 1.

Symmetry sharding: the NxN matrix is symmetric, so each core computes only
5 of 8 local 1024-column blocks (cols rotated so its own row-block is block
0). Blocks 0 (diagonal) and 4 (antipodal, computed by both pair cores)
contribute via row sums only; blocks 1-3 additionally produce per-column
sums (PE ones-matmul, accumulated over each 2-row-chunk group, row 0 of the
PSUM result DMA'd out) which the host adds to the row sums of the
corresponding other band. Elementwise work drops to 5/8.

Device per core:
  PE : s/pm matmuls (bf16), ones-matmul column sums of e and lc tiles
  DVE: evac x = C1*s + C0X (3/5 of chunks), z = x*x, d2 = x - g,
       lc = min(pm,1)*l2 (+row accum)
  ACT: evac (2/5), g = Sqrt(z - 1), l2 = Ln(d2), e = Exp(K*l2) (+row accum),
       batched per 2-row-chunk group by table set (sqrt | ln+exp)
Host: npos from label bincounts (exact); diagonal e_ii/l2_ii mirrored in
numpy with identical bf16 arithmetic; column-sum reassembly; final mean.
"""

import numpy as np
import ml_dtypes

import concourse.bass as bass
import concourse.tile as tile
import concourse.mybir as mybir
from concourse.bass_utils import run_bass_kernel_spmd

F32 = mybir.dt.float32
BF16 = mybir.dt.bfloat16
AX = mybir.AxisListType
OP = mybir.AluOpType
AF = mybir.ActivationFunctionType
BF = ml_dtypes.bfloat16

N = 8192
D = 128
NCORES = 8
RPC = N // NCORES        # 1024 rows per core
NRC = RPC // 128         # 8 row sub-chunks of 128
CCW = 512
NCP = 5                  # local 1024-col chunk-pairs computed (blocks 0-4)
NLC = NCP * 1024         # 5120 local columns
NCS = 6                  # column-sum strips of 512 (blocks 1-3)
CS0 = 1024               # first column with column sums
NOH = 48
GRC = 2                  # row sub-chunks per ACT-batching group
NGRP = NRC // GRC

C = 0.05
TEMP = 0.5
K = float(1.0 / (np.sqrt(C) * TEMP))          # 8.944271909999159
SD = 0.45125
C1 = float(np.float32(-0.1 / SD))              # x = C1*s + C0X
C0X = float(np.float32(0.1 / SD + 1.0))

# input blob layout (bf16)
OF_LF = 0
OF_RF = 1024
OF_LO = OF_RF + NLC
OF_RO = OF_LO + 1024
W_IN = OF_RO + NLC

# outs: [128, 0:8) se rowsums | [8:16) slc rowsums
# outs_cs: [1, grp*2*NCS*CCW] per-group colsums (e strips then lc strips)
W_CS = NGRP * 2 * NCS * CCW
_CACHE: dict = {}


class _SplitDrainTC(tile.TileContext):
    """TileContext whose kernel-tail drain is split into a chain of
    single-wait drains (walrus CTRL holds one sync wait)."""

    def _drain_and_barrier(self, tick_clock, wait_clock):
        from concourse.tile import ScopedClock

        d = self.nc.sync.drain()
        wait_clock.add_sem_waits(d.ins, ScopedClock({None: tick_clock.global_clock}))
        si = d.ins.sync_info
        waits = list(si.on_wait) if si is not None else []
        if len(waits) > 1:
            si.on_wait = waits[:1]
            for w in waits[1:]:
                d2 = self.nc.sync.drain()
                si2 = d2.ins.sync_info
                if si2 is None:
                    d2.ins.sync_info = mybir.SyncInfo(on_wait=[w], on_update=[])
                else:
                    si2.on_wait = [w]
        self.nc.all_engine_barrier()
        popped = self.nc._tile_sem_poison_stack.pop()
        assert popped is self._sem_poison
        self.nc.clear_and_free_semaphores(list(self.sems.allocated().values()))
        self.nc.all_engine_barrier()


def _build_nc():
    nc = bass.Bass()
    inp = nc.dram_tensor("inp", [D, W_IN], BF16, kind="ExternalInput")
    outs = nc.dram_tensor("outs", [128, 2 * NRC], F32, kind="ExternalOutput")
    outs_cs = nc.dram_tensor("outs_cs", [1, W_CS], F32, kind="ExternalOutput")

    with (
        _SplitDrainTC(nc) as tc,
        tc.tile_pool(name="const", bufs=1) as cpool,
        tc.tile_pool(name="mp", bufs=2) as mpool,
        tc.tile_pool(name="zp", bufs=2) as zpool,
        tc.tile_pool(name="gp", bufs=2) as gpool,
        tc.tile_pool(name="lp", bufs=3) as lpool,
        tc.tile_pool(name="ep", bufs=2) as epool,
        tc.tile_pool(name="lq", bufs=2) as lqpool,
        tc.tile_pool(name="acc", bufs=3) as apool,
        tc.tile_pool(name="lcw", bufs=3) as lcpool,
        tc.tile_pool(name="fen", bufs=2) as fenpool,
        tc.tile_pool(name="st", bufs=2) as stpool,
        tc.tile_pool(name="ps", bufs=2, space="PSUM") as pspool,
        tc.tile_pool(name="pm", bufs=2, space="PSUM") as pmpool,
        tc.tile_pool(name="cs", bufs=2, space="PSUM") as cspool,
    ):
        inps = cpool.tile([D, W_IN], BF16)
        # stage the input so the first s-matmuls start as early as possible
        nc.gpsimd.dma_start(inps[:, 0:3072], inp[:, 0:3072])
        nc.gpsimd.dma_start(inps[:, 3072:OF_LO], inp[:, 3072:OF_LO])
        nc.gpsimd.dma_start(inps[:, OF_LO:W_IN], inp[:, OF_LO:W_IN])

        bias_s = cpool.tile([128, 1], F32)
        nc.vector.memset(bias_s[:], -1.0)
        ones = cpool.tile([128, 1], BF16)
        nc.vector.memset(ones[:], 1.0)

        fin = cpool.tile([128, 2 * NRC], F32)

        def lhs_f(rc):
            return inps[:, OF_LF + rc * 128:OF_LF + (rc + 1) * 128]

        def rhs_f(c0, c1):
            return inps[:, OF_RF + c0:OF_RF + c1]

        def lhs_oh(rc):
            return inps[0:NOH, OF_LO + rc * 128:OF_LO + (rc + 1) * 128]

        def rhs_oh(c0, c1):
            return inps[0:NOH, OF_RO + c0:OF_RO + c1]

        def emit_pm_lc_cs(grp, rcs, lts, ets):
            # DVE fence on the last Ln output so lc STTs carry one wait.
            fence = fenpool.tile([128, 1], BF16, tag="fence")
            nc.vector.tensor_copy(fence[:], lts[rcs[-1]][:, 0:1])
            lqs = {}
            for rc in rcs:
                lq = lqpool.tile([128, NCS * CCW], BF16, tag="lq")
                lqs[rc] = lq
                acc_lc = apool.tile([128, 2 * NCP], F32, tag="acc_lc")
                for cc in range(2 * NCP):
                    pm = pmpool.tile([128, CCW], F32, tag="pm")
                    nc.tensor.matmul(pm[:], lhs_oh(rc),
                                     rhs_oh(cc * CCW, (cc + 1) * CCW),
                                     start=True, stop=True)
                    if 2 <= cc < 8:        # cols [1024,4096): keep product
                        lco = lq[:, (cc - 2) * CCW:(cc - 1) * CCW]
                    else:
                        lcot = lcpool.tile([128, CCW], BF16, tag="lc")
                        lco = lcot[:]
                    nc.vector.scalar_tensor_tensor(
                        lco, pm[:], 1.0,
                        lts[rc][:, cc * CCW:(cc + 1) * CCW],
                        OP.min, OP.mult,
                        accum_out=acc_lc[:, cc:cc + 1])
                nc.vector.reduce_sum(fin[:, NRC + rc:NRC + rc + 1],
                                     acc_lc[:], axis=AX.X)
            # column sums for blocks 1-3, accumulated over the group's rcs
            base = grp * 2 * NCS * CCW
            stage = stpool.tile([1, 2 * NCS * CCW], F32, tag="stage")
            for k in range(NCS):
                cse = cspool.tile([1, CCW], F32, tag="cs")
                for j, rc in enumerate(rcs):
                    nc.tensor.matmul(
                        cse[:], ones[:, 0:1],
                        ets[rc][:, CS0 + k * CCW:CS0 + (k + 1) * CCW],
                        start=(j == 0), stop=(j == len(rcs) - 1))
                nc.scalar.activation(stage[:, k * CCW:(k + 1) * CCW],
                                     cse[:], AF.Copy)
            for k in range(NCS):
                csl = cspool.tile([1, CCW], F32, tag="cs")
                for j, rc in enumerate(rcs):
                    nc.tensor.matmul(
                        csl[:], ones[:, 0:1],
                        lqs[rc][:, k * CCW:(k + 1) * CCW],
                        start=(j == 0), stop=(j == len(rcs) - 1))
                nc.vector.tensor_copy(
                    stage[:, (NCS + k) * CCW:(NCS + k + 1) * CCW], csl[:])
            nc.gpsimd.dma_start(
                outs_cs[:, base:base + 2 * NCS * CCW], stage[:])

        prev = None
        for grp in range(NGRP):
            rcs = tuple(range(grp * GRC, (grp + 1) * GRC))
            mts, zts, gts, lts, ets = {}, {}, {}, {}, {}
            for rc in rcs:
                mt = mpool.tile([128, NLC], BF16, tag="m")
                mts[rc] = mt
                for cp in range(NCP):
                    ps = pspool.tile([128, 2 * CCW], F32, tag="ps")
                    nc.tensor.matmul(ps[:, 0:CCW], lhs_f(rc),
                                     rhs_f(cp * 1024, cp * 1024 + CCW),
                                     start=True, stop=True)
                    nc.tensor.matmul(ps[:, CCW:2 * CCW], lhs_f(rc),
                                     rhs_f(cp * 1024 + CCW, (cp + 1) * 1024),
                                     start=True, stop=True)
                    dst = mt[:, cp * 1024:(cp + 1) * 1024]
                    if cp >= 3:
                        nc.scalar.activation(dst, ps[:], AF.Copy,
                                             bias=C0X, scale=C1)
                    else:
                        nc.vector.tensor_scalar(dst, ps[:], C1, C0X,
                                                OP.mult, OP.add)
                zt = zpool.tile([128, NLC], BF16, tag="z")
                zts[rc] = zt
                nc.vector.tensor_tensor(zt[:], mt[:], mt[:], OP.mult)
            if prev is not None:
                emit_pm_lc_cs(*prev)
            for rc in rcs:
                gt = gpool.tile([128, NLC], BF16, tag="g")
                gts[rc] = gt
                nc.scalar.activation(gt[:], zts[rc][:], AF.Sqrt,
                                     bias=bias_s[:])          # g = sqrt(z-1)
            for rc in rcs:       # d2 = x - g (reuse z tile)
                nc.vector.tensor_tensor(zts[rc][:], mts[rc][:], gts[rc][:],
                                        OP.subtract)
            for rc in rcs:
                lt = lpool.tile([128, NLC], BF16, tag="l")
                lts[rc] = lt
                nc.scalar.activation(lt[:], zts[rc][:], AF.Ln)
            for rc in rcs:       # e = exp(K*l2), row-accumulated
                et = epool.tile([128, NLC], BF16, tag="e")
                ets[rc] = et
                nc.scalar.activation(et[:], lts[rc][:], AF.Exp, scale=K,
                                     accum_out=fin[:, rc:rc + 1])
            prev = (grp, rcs, lts, ets)
        emit_pm_lc_cs(*prev)

        nc.gpsimd.dma_start(outs[:], fin[:])

    _split_multi_waits(nc)
    return nc


def _split_multi_waits(nc):
    """Walrus CTRL encodings hold a single sync wait. For any instruction
    carrying more, peel the extra waits onto same-engine drain instructions
    inserted immediately before it (same queue position -> identical
    semantics)."""
    ctr = 0
    for bbh in nc.bb_map.values():
        bb = bbh.bb if hasattr(bbh, "bb") else bbh
        il = list(bb.instructions)
        out = []
        changed = False
        for ins in il:
            si = ins.sync_info
            waits = list(si.on_wait) if si is not None else []
            if len(waits) > 1:
                changed = True
                for w in waits[1:]:
                    d = mybir.InstDrain(name=f"wsplit{ctr}", ins=[], outs=[])
                    ctr += 1
                    d.engine = ins.engine
                    d.sync_info = mybir.SyncInfo(on_wait=[w], on_update=[])
                    nc.register_instruction(d, overwrite=True)
                    out.append(d)
                si.on_wait = waits[:1]
            out.append(ins)
        if changed:
            bb.instructions = out


def _get_nc():
    if "nc" not in _CACHE:
        _CACHE["nc"] = _build_nc()
    return _CACHE["nc"]


def _bf(x):
    return np.asarray(x, dtype=BF).astype(np.float32)


def kernel(features, primary_labels, secondary_labels):
    features = np.asarray(features, dtype=np.float32)
    pl = np.asarray(primary_labels).astype(np.int64)
    sl = np.asarray(secondary_labels).astype(np.int64)

    nrm = np.maximum(np.linalg.norm(features, axis=1, keepdims=True), 1e-12)
    f = (features / nrm).astype(np.float32)

    lhs_bf = _bf(f)
    eta = 0.0
    for _ in range(6):
        rhs_bf = _bf((1.0 - eta) * f)
        diag = np.einsum("nd,nd->n", lhs_bf, rhs_bf).astype(np.float32)
        if diag.max() <= 1.0 - 3e-4:
            break
        eta = eta + (float(diag.max()) - (1.0 - 4e-4))

    lhsT = np.ascontiguousarray(lhs_bf.T).astype(BF)
    rhsT = np.ascontiguousarray(rhs_bf.T).astype(BF)

    oh = np.zeros((NOH, N), dtype=BF)
    oh[pl, np.arange(N)] = 1.0
    oh[32 + sl, np.arange(N)] = 1.0

    in_maps = []
    for c in range(NCORES):
        shift = c * RPC
        lr = np.roll(lhsT, -shift, axis=1)
        rr = np.roll(rhsT, -shift, axis=1)
        orr = np.roll(oh, -shift, axis=1)
        buf = np.zeros((D, W_IN), dtype=BF)
        buf[:, OF_LF:OF_LF + 1024] = lr[:, 0:1024]
        buf[:, OF_RF:OF_RF + NLC] = rr[:, 0:NLC]
        buf[0:NOH, OF_LO:OF_LO + 1024] = orr[:, 0:1024]
        buf[0:NOH, OF_RO:OF_RO + NLC] = orr[:, 0:NLC]
        in_maps.append({"inp": buf})

    nc = _get_nc()
    res = run_bass_kernel_spmd(nc, in_maps, list(range(NCORES)))
    results = res.results

    se = np.zeros(N, np.float64)
    slc2 = np.zeros(N, np.float64)
    for c in range(NCORES):
        r = results[c]
        for rc in range(NRC):
            g0 = c * RPC + rc * 128
            se[g0:g0 + 128] += r["outs"][:, rc]
            slc2[g0:g0 + 128] += r["outs"][:, NRC + rc]
        # column sums: local cols [1024, 4096) -> global rows of bands c+1..c+3
        cs = r["outs_cs"][0].astype(np.float64)   # [NGRP * 2 * NCS * CCW]
        cs = cs.reshape(NGRP, 2, NCS * CCW)
        cse = cs[:, 0, :].sum(axis=0)             # [3072] over row groups
        csl = cs[:, 1, :].sum(axis=0)
        jg = (np.arange(CS0, CS0 + NCS * CCW) + c * RPC) % N
        se[jg] += cse
        slc2[jg] += csl

    # diagonal mirror (device bf16 arithmetic replicated on host)
    x_ii = _bf(np.float32(C1) * diag + np.float32(C0X))
    z_ii = _bf(x_ii * x_ii)
    g_ii = _bf(np.sqrt(z_ii - np.float32(1.0)))
    d2_ii = _bf(x_ii - g_ii)
    l2_ii = _bf(np.log(d2_ii)).astype(np.float64)
    e_ii = np.exp(np.float32(K) * l2_ii.astype(np.float32)).astype(np.float64)

    cnt_p = np.bincount(pl, minlength=32)
    cnt_s = np.bincount(sl, minlength=16)
    comb = pl * 16 + sl
    cnt_ps = np.bincount(comb, minlength=512)
    npos = (cnt_p[pl] + cnt_s[sl] - cnt_ps[comb] - 1).astype(np.float64)

    denom = se - e_ii + 1e-8
    row_sum = K * (slc2 - l2_ii) - np.log(denom) * npos
    valid = npos > 0
    per_row = np.where(valid, row_sum / np.maximum(npos, 1.0), 0.0)
    n_valid = valid.sum()
    loss = -per_row.sum() / max(n_valid, 1) * TEMP if n_valid > 0 else 0.0
    loss = np.nan_to_num(np.float32(loss), nan=0.0, posinf=0.0, neginf=0.0)
    return np.float32(loss)
